# revision 16
# baseline (speedup 1.0000x reference)
"""Trainium2 Bass kernel for the DTW mask calculator.

Computes, for N=8192:
    out = where(sd < 5, exp(-sd^2), 0) * where(labels[i]==labels[j], 1, 0.1)
          * exp(-dtw^2)

Row-sharded across 8 NeuronCores (1024 rows each). adj_mx is unused by the
reference computation and never uploaded.

The dominant cost in this deployment is per-dispatch operand staging in the
axon/PJRT runtime (~11.4 GB/s over all operand bytes, plus ~20 ms fixed
round-trip), not device execution (~0.15 ms at the 358 GB/s/core HBM
roofline). So the kernel is designed to minimise operand bytes:
  - sd/dtw are uploaded as float16 (verified 4e-4 norm rel err end-to-end,
    vs the 2e-2 gate) and the output is written/fetched as float16 and
    upcast on the host.
  - cluster labels go up as a [1, 8192] f16 row per core (16 KB) and are
    broadcast across partitions on-device (gpsimd partition_broadcast),
    instead of a pre-broadcast [128, 8192] operand.
  - the dispatch binds the bass_exec primitive directly without the
    zero-filled output operand that run_bass_via_pjrt passes for its
    donation trick; this kernel writes every output element, so the
    custom-call result buffer needs no initialisation, and dropping the
    operand removes its per-dispatch staging cost. Otherwise the dispatch
    mirrors concourse.bass2jax.run_bass_via_pjrt (the same path
    bass_utils.run_bass_kernel_spmd redirects to under axon).

Per [128, 4096] chunk (in-place where a tile is dead afterwards):
  ACT: z1 = Square(sd); z1 <- Exp(-z1) after the add
  DVE: dtw <- dtw*dtw; z1 <- z1+dtw; aext = max(lcol==lrow, 0.1)
       [dual-op tensor_scalar]; sd <- (sd<5)*exp [fused
       scalar_tensor_tensor]; out = sd*aext
(GpSimd was tried for the add/compare and is ~10x slower than DVE for
bulk elementwise on this part -- keep it to the one-time label
partition_broadcast only.)
"""

import numpy as np

N = 8192
N_CORES = 8
R = N // N_CORES          # rows per core = 1024
P = 128                   # partitions
RT = R // P               # row tiles per core = 8
W = 4096                  # column chunk width
CT = N // W               # column chunks

# Fallback switches (flipped only if the lean path fails in this stack):
PASS_OUT_OPERAND = False  # True: pass the zero 'out' operand like
                          # run_bass_via_pjrt does
ONDEVICE_LAB_BCAST = True   # ship labels as [1, N] + on-device
                            # partition_broadcast; the pre-broadcast
                            # [128, N] operand variant measured no faster
                            # (the broadcast hides behind the first DMAs)

_CACHE = {}


def _build(passes=1, w=None, bufs=3):
    import concourse.tile as tile
    from concourse import bacc, mybir

    f16 = mybir.dt.float16
    f32 = mybir.dt.float32
    AF = mybir.ActivationFunctionType
    OP = mybir.AluOpType

    nc = bacc.Bacc("TRN2", target_bir_lowering=False, debug=False,
                   num_devices=N_CORES)

    sd = nc.dram_tensor("sd", [R, N], f16, kind="ExternalInput").ap()
    dtw = nc.dram_tensor("dtw", [R, N], f16, kind="ExternalInput").ap()
    if ONDEVICE_LAB_BCAST:
        lab = nc.dram_tensor("lab", [1, N], f16, kind="ExternalInput").ap()
    else:
        lab = nc.dram_tensor("lab", [P, N], f16, kind="ExternalInput").ap()
    lrow = nc.dram_tensor("lrow", [P, RT], f32, kind="ExternalInput").ap()
    out = nc.dram_tensor("out", [R, N], f16, kind="ExternalOutput").ap()

    w = W if w is None else w
    ct = N // w
    with tile.TileContext(nc) as tc:
        with (
            tc.tile_pool(name="const", bufs=1) as const,
            tc.tile_pool(name="io", bufs=bufs) as io,
            tc.tile_pool(name="tmp", bufs=bufs) as tmp,
        ):
            lcol_t = const.tile([P, N], f16)
            if ONDEVICE_LAB_BCAST:
                lab_t = const.tile([1, N], f16)
                nc.sync.dma_start(lab_t[:], lab[:, :])
                nc.gpsimd.partition_broadcast(lcol_t[:], lab_t[:])
            else:
                nc.sync.dma_start(lcol_t[:], lab[:, :])
            lrow_t = const.tile([P, RT], f32)
            nc.sync.dma_start(lrow_t[:], lrow[:, :])

            for _ in range(passes):
                for rt in range(RT):
                    rs = slice(rt * P, (rt + 1) * P)
                    for c in range(ct):
                        cs = slice(c * w, (c + 1) * w)
                        sd_t = io.tile([P, w], f16, tag="sd")
                        nc.sync.dma_start(sd_t[:], sd[rs, cs])
                        dtw_t = io.tile([P, w], f16, tag="dtw")
                        nc.sync.dma_start(dtw_t[:], dtw[rs, cs])

                        # z1 = sd^2 (ACT); dtw <- dtw^2, z1 <- z1+dtw^2 (DVE,
                        # in place); z1 <- exp(-z1) (ACT, in place)
                        z1_t = tmp.tile([P, w], f16, tag="z1")
                        nc.scalar.activation(z1_t[:], sd_t[:], AF.Square)
                        nc.vector.tensor_mul(dtw_t[:], dtw_t[:], dtw_t[:])
                        nc.vector.tensor_add(z1_t[:], z1_t[:], dtw_t[:])
                        nc.scalar.activation(z1_t[:], z1_t[:], AF.Exp,
                                             scale=-1.0)

                        aext_t = tmp.tile([P, w], f16, tag="aext")
                        nc.vector.tensor_scalar(
                            aext_t[:], lcol_t[:, cs], lrow_t[:, rt:rt + 1],
                            0.1, op0=OP.is_equal, op1=OP.max,
                        )
                        # sd <- (sd < 5) * exp(...) (in place)
                        nc.vector.scalar_tensor_tensor(
                            sd_t[:], sd_t[:], 5.0, z1_t[:],
                            op0=OP.is_lt, op1=OP.mult,
                        )
                        out_t = io.tile([P, w], f16, tag="out")
                        nc.vector.tensor_mul(out_t[:], sd_t[:], aext_t[:])
                        nc.sync.dma_start(out[rs, cs], out_t[:])

    nc.compile()
    return nc


def _get_exec(passes=1, w=None, bufs=3):
    """Build (once) and return (jitted_fn, mesh, partition_name).

    The jitted fn mirrors bass2jax.run_bass_via_pjrt's shard_map dispatch,
    minus the donated zero output operand (see module docstring).
    passes>1 repeats the whole compute in one NEFF (used by probes to
    separate device pass time from per-dispatch overhead).
    """
    key = ("exec", passes, w, bufs)
    if key in _CACHE:
        return _CACHE[key]

    import jax
    from jax.sharding import Mesh, PartitionSpec
    from jax.experimental.shard_map import shard_map
    from concourse import bass2jax

    nckey = ("nc", passes, w, bufs)
    if nckey not in _CACHE:
        _CACHE[nckey] = _build(passes, w, bufs)
    nc = _CACHE[nckey]
    bass2jax.install_neuronx_cc_hook()

    out_aval = jax.core.ShapedArray((R, N), np.float16)
    pn = nc.partition_id_tensor.name if nc.partition_id_tensor else None
    in_names = ("sd", "dtw", "lab", "lrow")
    bind_in_names = in_names + (("out",) if PASS_OUT_OPERAND else ())
    if pn is not None:
        bind_in_names = bind_in_names + (pn,)

    def _body(*args):
        operands = list(args)
        if pn is not None:
            operands.append(bass2jax.partition_id_tensor())
        outs = bass2jax._bass_exec_p.bind(
            *operands,
            out_avals=(out_aval,),
            in_names=bind_in_names,
            out_names=("out",),
            lowering_input_output_aliases=(),
            sim_require_finite=True,
            sim_require_nnan=True,
            nc=nc,
        )
        return tuple(outs)

    devices = jax.devices()[:N_CORES]
    mesh = Mesh(np.asarray(devices), ("core",))
    n_in = len(in_names) + (1 if PASS_OUT_OPERAND else 0)

    def _mk_jit():
        return jax.jit(shard_map(
            _body, mesh=mesh,
            in_specs=(PartitionSpec("core"),) * n_in,
            out_specs=(PartitionSpec("core"),),
            check_rep=False,
        ))

    # AOT-compile with bass_effect suppressed -> C++ fast-path dispatch
    # (lower per-call host overhead). Fall back to plain jit if the AOT
    # path errors in this environment.
    from jax.sharding import NamedSharding
    sh = NamedSharding(mesh, PartitionSpec("core"))
    aot_args = [
        jax.ShapeDtypeStruct((N, N), np.float16, sharding=sh),
        jax.ShapeDtypeStruct((N, N), np.float16, sharding=sh),
        jax.ShapeDtypeStruct(
            ((N_CORES, N) if ONDEVICE_LAB_BCAST else (N_CORES * P, N)),
            np.float16, sharding=sh),
        jax.ShapeDtypeStruct((N_CORES * P, RT), np.float32, sharding=sh),
    ]
    if PASS_OUT_OPERAND:
        aot_args.append(jax.ShapeDtypeStruct((N, N), np.float16, sharding=sh))
    try:
        fn = bass2jax.fast_dispatch_compile(
            lambda: _mk_jit().lower(*aot_args).compile())
    except Exception as e:
        import sys
        print(f"fast_dispatch_compile failed ({e!r}); using plain jit",
              file=sys.stderr)
        fn = _mk_jit()
    _CACHE[key] = (fn, mesh, pn)
    return _CACHE[key]


def _prep_host_args(sd_mx, dtw_matrix, cluster_labels):
    """Global (concatenated-over-cores) host arrays in dispatch order."""
    sd16 = np.asarray(sd_mx).astype(np.float16)
    dtw16 = np.asarray(dtw_matrix).astype(np.float16)
    lab16 = np.asarray(cluster_labels).astype(np.float16)
    # per-core [1, N] rows, concatenated -> [N_CORES, N]
    lab_g = np.ascontiguousarray(
        np.broadcast_to(lab16[None, :], (N_CORES, N))
        if ONDEVICE_LAB_BCAST else
        np.broadcast_to(lab16[None, :], (N_CORES * P, N)))
    # per-core [P, RT] with lrow[p, rt] = labels[r0 + rt*P + p]
    labf = np.asarray(cluster_labels).astype(np.float32)
    lrow_g = np.ascontiguousarray(
        labf.reshape(N_CORES, RT, P).transpose(0, 2, 1).reshape(N_CORES * P, RT))
    args = [sd16, dtw16, lab_g, lrow_g]
    if PASS_OUT_OPERAND:
        args.append(np.zeros((N, N), np.float16))
    return args


def _put_args(host_args, mesh):
    import jax
    from jax.sharding import NamedSharding, PartitionSpec
    sh = NamedSharding(mesh, PartitionSpec("core"))
    return [jax.device_put(a, sh) for a in host_args]


def kernel(adj_mx, sd_mx, dtw_matrix, cluster_labels):
    fn, mesh, _ = _get_exec()
    host_args = _prep_host_args(sd_mx, dtw_matrix, cluster_labels)
    args = _put_args(host_args, mesh)
    (out16,) = fn(*args)
    return np.asarray(out16).astype(np.float32)
